# revision 1
# baseline (speedup 1.0000x reference)
"""GATv2 (2-layer, 8-head) message-passing kernel for Trainium2, 8 NeuronCores.

Strategy (sharding_hint: partition edges by destination, nodes by range):
- Host: nodes split into 8 contiguous ranges (12544/core incl. padding); within a
  core nodes are ordered by degree (desc) and packed into 98 groups of 128
  destination nodes. Edges of each node occupy a fixed-width row ("W-grid") in
  its group; group width Wg = cross-core max degree in that group (degree
  sorting makes padding tight). Per-edge gather indices are precomputed.
- Device, per core: project x -> h (PE); per layer compute xl/xr for owned
  nodes (PE), AllGather the xl table (edge sources span all cores), then for
  each group: prefill tile with xr (dst) broadcast, indirect-DMA gather of
  xl[src] with CCE add (s = xl[src]+xr[dst]), leaky-relu / att dot / softmax /
  weighted aggregation with free-dim reduces (DVE/ACT). Softmax max-subtraction
  is skipped (logits are O(1); softmax is shift-invariant). Pad slots gather a
  poison row whose value makes their logits <= -30 so exp()==0 exactly.
- The aggregated numerator uses s (= xl+xr) instead of xl; the xr*denominator
  surplus is subtracted per node afterwards.
"""

import math
from dataclasses import dataclass, field

import numpy as np

# ---- problem constants (hardcoded; harness calls kernel(**inputs) directly) --
N = 100000
E = 3200000
IN_C = 1433
DIM = 32
HEADS = 8
OUT_C = 4
NUM_LAYER = 2
NUM_CLASS = 7
NEG_SLOPE = 0.2
NCORES = 8
BIG = 3.0e4


@dataclass
class Cfg:
    ncores: int
    n_real: int
    nl: int           # owned (padded) nodes per core, = ngrp*128
    ngrp: int
    inp: int          # padded input feature dim (multiple of 128, >= IN_C+1)
    in_c: int
    d: int            # DIM
    h: int            # HEADS
    c: int            # OUT_C
    ncls: int
    nlayer: int
    wg: tuple         # per-group W
    cumw: tuple
    sw: int
    wmax: int
    npad: int
    padrow: int
    debug: bool = False


def _ceil_to(x, m):
    return (x + m - 1) // m * m


def host_prep(x, edge_index, w_proj, b_proj, w_l, b_l, w_r, b_r, att, conv_bias,
              w_pred, b_pred, ncores):
    """Numpy preprocessing: node permutation, W-grid gather indices, padded /
    transposed inputs. Returns (cfg, per_core_inputs, shared_inputs, meta)."""
    x = np.asarray(x)
    edge_index = np.asarray(edge_index)
    n_real, in_c = x.shape
    d = w_proj.shape[1]
    h, c = att.shape[1], att.shape[2]
    nlayer = w_l.shape[0]
    ncls = w_pred.shape[1]
    e = edge_index.shape[1]

    nl = _ceil_to(_ceil_to(n_real, ncores) // ncores, 128)
    ngrp = nl // 128
    npad = nl * ncores
    assert n_real < npad, "need at least one fake node for the pad row"

    src = edge_index[0].astype(np.int64)
    dst = edge_index[1].astype(np.int64)
    deg = np.bincount(dst, minlength=npad)
    assert deg[:n_real].min() >= 1, "zero-degree real node breaks the xr-surplus trick"

    # per-core degree-sorted permutation; local id = degree rank = g*128 + p
    g2pg = np.empty(npad, np.int64)
    nodes_by_lid = np.empty((ncores, nl), np.int64)
    for k in range(ncores):
        gn = np.arange(k * nl, (k + 1) * nl)
        order = np.argsort(-deg[gn], kind="stable")
        nodes = gn[order]                          # node at rank r == lid r
        g2pg[nodes] = k * nl + np.arange(nl)
        nodes_by_lid[k] = nodes

    sorted_deg = -np.sort(-deg.reshape(ncores, nl), axis=1)
    grp_max = sorted_deg.reshape(ncores, ngrp, 128).max(axis=2).max(axis=0)
    wg = np.maximum(_ceil_to(grp_max, 4), 4).astype(np.int64)
    cumw = np.concatenate([[0], np.cumsum(wg)])
    sw = int(cumw[-1])
    padrow = npad - 1
    assert deg[nodes_by_lid[ncores - 1, nl - 1]] == 0 or nodes_by_lid[ncores - 1, nl - 1] >= n_real

    # pad-row poison: V_c = -sign(att_l[c]) * BIG must give very negative logits
    att_flat = att.reshape(nlayer, h * c)
    if nlayer:
        min_att_sum = np.abs(att_flat).reshape(nlayer, h, c).sum(-1).min()
        assert min_att_sum * NEG_SLOPE * BIG > 30, f"pad poison too weak: {min_att_sum}"

    # edge grids
    dpg = g2pg[dst]
    spg = g2pg[src]
    eo = np.argsort(dpg, kind="stable")
    dpg_s = dpg[eo]
    spg_s = spg[eo]
    _, starts, counts = np.unique(dpg_s, return_index=True, return_counts=True)
    j = np.arange(e) - np.repeat(starts, counts)
    k_e = dpg_s // nl
    lid_e = dpg_s % nl
    p_e = lid_e % 128
    g_e = lid_e // 128
    col_e = cumw[g_e] + j
    assert (j < wg[g_e]).all()
    idx_all = np.full((ncores, 128, sw), padrow, np.int32)
    idx_all[k_e, p_e, col_e] = spg_s.astype(np.int32)

    # x, transposed + padded, with ones row for the bias trick
    inp = _ceil_to(in_c + 1, 128)
    per_core = []
    for k in range(ncores):
        xt = np.zeros((inp, nl), np.float32)
        nodes = nodes_by_lid[k]
        real = nodes < n_real
        xt[:in_c, real] = x[nodes[real]].T
        xt[in_c, :] = 1.0
        per_core.append({"x_t": xt, "idx_all": np.ascontiguousarray(idx_all[k])})

    wmax = int(wg.max())
    wp_pad = np.zeros((inp, d), np.float32)
    wp_pad[:in_c] = w_proj
    wp_pad[in_c] = b_proj
    shared = {"w_proj": wp_pad, "w_pred": np.vstack([w_pred, b_pred[None, :]]).astype(np.float32)}
    for l in range(nlayer):
        shared[f"wl{l}"] = np.vstack([w_l[l], b_l[l][None, :]]).astype(np.float32)
        shared[f"wr{l}"] = np.vstack([w_r[l], b_r[l][None, :]]).astype(np.float32)
        shared[f"att{l}"] = np.broadcast_to(
            att_flat[l][None, None, :], (128, wmax, h * c)).reshape(128, wmax * h * c).astype(np.float32)
        shared[f"cb{l}"] = np.broadcast_to(conv_bias[l][None, :], (128, h * c)).astype(np.float32)
        shared[f"padv{l}"] = (-np.sign(att_flat[l]) * BIG).astype(np.float32)[None, :]

    cfg = Cfg(ncores=ncores, n_real=n_real, nl=nl, ngrp=ngrp, inp=inp, in_c=in_c,
              d=d, h=h, c=c, ncls=ncls, nlayer=nlayer, wg=tuple(int(w) for w in wg),
              cumw=tuple(int(w) for w in cumw), sw=sw, wmax=wmax, npad=npad,
              padrow=padrow)
    meta = {"nodes_by_lid": nodes_by_lid}
    return cfg, per_core, shared, meta


def build_program(cfg: Cfg):
    import concourse.bass as bass
    import concourse.bacc as bacc
    import concourse.mybir as mybir
    import concourse.tile as tile
    from concourse.masks import make_identity
    from concourse.tile import add_dep_helper

    f32 = mybir.dt.float32
    i32 = mybir.dt.int32
    P = 128
    D, H, C = cfg.d, cfg.h, cfg.c
    NGRP, NL = cfg.ngrp, cfg.nl
    HS = D + 1  # h chunk stride (extra ones column for the bias-row trick)

    nc = bacc.Bacc(trn_type="TRN2", num_devices=cfg.ncores)

    x_t = nc.dram_tensor("x_t", [cfg.inp, NL], f32, kind="ExternalInput")
    idx_in = nc.dram_tensor("idx_all", [P, cfg.sw], i32, kind="ExternalInput")
    wp_in = nc.dram_tensor("w_proj", [cfg.inp, D], f32, kind="ExternalInput")
    wpred_in = nc.dram_tensor("w_pred", [D + 1, cfg.ncls], f32, kind="ExternalInput")
    wl_in = [nc.dram_tensor(f"wl{l}", [D + 1, D], f32, kind="ExternalInput") for l in range(cfg.nlayer)]
    wr_in = [nc.dram_tensor(f"wr{l}", [D + 1, D], f32, kind="ExternalInput") for l in range(cfg.nlayer)]
    att_in = [nc.dram_tensor(f"att{l}", [P, cfg.wmax * D], f32, kind="ExternalInput") for l in range(cfg.nlayer)]
    cb_in = [nc.dram_tensor(f"cb{l}", [P, D], f32, kind="ExternalInput") for l in range(cfg.nlayer)]
    padv_in = [nc.dram_tensor(f"padv{l}", [1, D], f32, kind="ExternalInput") for l in range(cfg.nlayer)]
    out_dram = nc.dram_tensor("out", [P, NGRP * cfg.ncls], f32, kind="ExternalOutput")
    dbg_h = (nc.dram_tensor("dbg_h", [P, NGRP * (D + 1)], f32, kind="ExternalOutput")
             if cfg.debug else None)

    xl_own = nc.dram_tensor("xl_own", [NL, D], f32)
    # collective output (Shared scratchpad); gathers read the Local copy
    # (indirect DMA against the runtime-relocated Shared base is risky)
    if cfg.ncores > 4:  # shared-output collectives need >4 cores
        xl_allg = nc.dram_tensor("xl_allg", [cfg.npad, D], f32, addr_space="Shared")
    else:
        xl_allg = nc.dram_tensor("xl_allg", [cfg.npad, D], f32)
    xl_all = nc.dram_tensor("xl_all", [cfg.npad, D], f32)

    with tile.TileContext(nc) as tc:
        with (
            tc.tile_pool(name="const", bufs=1) as cp,
            tc.tile_pool(name="pers", bufs=1) as pp,
            tc.tile_pool(name="work", bufs=3) as wp,
            tc.tile_pool(name="edge", bufs=2) as ep,
            tc.tile_pool(name="ps_mm", bufs=2, space="PSUM") as pmm,
            tc.tile_pool(name="ps_tr", bufs=2, space="PSUM") as ptr,
        ):
            # ---- constants -> SBUF ----
            ident = cp.tile([P, P], f32)
            make_identity(nc, ident[:])
            nj = cfg.inp // P
            wp_sb = cp.tile([P, nj * D], f32)
            # one DMA: row j*128+p of w_proj -> partition p, cols j*D..(j+1)*D
            nc.sync.dma_start(out=wp_sb[:].rearrange("p (j d) -> p j d", d=D),
                              in_=wp_in[:].rearrange("(j p) d -> p j d", p=P))
            wl_sb = [cp.tile([D + 1, D], f32, name=f"wl_sb{l}") for l in range(cfg.nlayer)]
            wr_sb = [cp.tile([D + 1, D], f32, name=f"wr_sb{l}") for l in range(cfg.nlayer)]
            cb_sb = [cp.tile([P, D], f32, name=f"cb_sb{l}") for l in range(cfg.nlayer)]
            for l in range(cfg.nlayer):
                nc.sync.dma_start(out=wl_sb[l][:], in_=wl_in[l][:])
                nc.sync.dma_start(out=wr_sb[l][:], in_=wr_in[l][:])
                nc.sync.dma_start(out=cb_sb[l][:], in_=cb_in[l][:])
            wpred_sb = cp.tile([D + 1, cfg.ncls], f32)
            nc.sync.dma_start(out=wpred_sb[:], in_=wpred_in[:])
            att_sb = pp.tile([P, cfg.wmax * D], f32)   # reloaded per layer
            ones_sb = cp.tile([P, 1], f32)
            nc.gpsimd.memset(ones_sb[:], 1.0)

            h_a = pp.tile([P, NGRP * HS], f32, name="h_a")
            h_b = pp.tile([P, NGRP * HS], f32, name="h_b")
            xr_own = pp.tile([P, NGRP * D], f32)

            def h_view(t):  # [P, NGRP, D] data columns
                return t[:].rearrange("p (g s) -> p g s", s=HS)[:, :, :D]

            def ones_col(t):
                return t[:].rearrange("p (g s) -> p g s", s=HS)[:, :, D:HS]

            # ---- P1: h0 = x @ w_proj + b_proj ----
            # column tiles of NL
            col_tiles = []
            c0 = 0
            while c0 < NL:
                tw = min(512, NL - c0)
                col_tiles.append((c0, tw))
                c0 += tw
            for (c0, tw) in col_tiles:
                h_acc = pmm.tile([D, 512], f32, tag="h_acc")
                for jj in range(nj):
                    xtile = wp.tile([P, 512], f32, tag="xtile")
                    nc.sync.dma_start(out=xtile[:, :tw], in_=x_t[jj * P:(jj + 1) * P, c0:c0 + tw])
                    nc.tensor.matmul(out=h_acc[:, :tw], lhsT=wp_sb[:, jj * D:(jj + 1) * D],
                                     rhs=xtile[:, :tw], start=(jj == 0), stop=(jj == nj - 1))
                hT_stage = wp.tile([D, 512], f32, tag="hT_stage")
                nc.scalar.copy(out=hT_stage[:, :tw], in_=h_acc[:, :tw])
                for t2 in range(tw // P):
                    ch = (c0 + t2 * P) // P
                    htr = ptr.tile([P, D], f32, tag="htr", bufs=1)
                    nc.tensor.transpose(out=htr[:], in_=hT_stage[:, t2 * P:(t2 + 1) * P],
                                        identity=ident[:D, :D])
                    nc.vector.tensor_copy(out=h_view(h_a)[:, ch, :], in_=htr[:])
            nc.vector.tensor_copy(out=ones_col(h_a)[:, :, 0], in_=ones_sb[:].to_broadcast([P, NGRP]))

            h_cur, h_nxt = h_a, h_b

            # ---- P2: layers ----
            for l in range(cfg.nlayer):
                nc.sync.dma_start(out=att_sb[:], in_=att_in[l][:])
                # (a) xl/xr for owned nodes; xl -> DRAM (+allgather), xr -> SBUF
                xl_dmas = []
                nbatch = math.ceil(NGRP / 16)
                for b in range(nbatch):
                    chunks = range(b * 16, min((b + 1) * 16, NGRP))
                    hT_chs = {}
                    for ch in chunks:
                        tr = ptr.tile([HS, P], f32, tag="tr")
                        nc.tensor.transpose(
                            out=tr[:], in_=h_cur[:].rearrange("p (g s) -> p g s", s=HS)[:, ch, :],
                            identity=ident[:])
                        hT_ch = wp.tile([HS, P], f32, tag="hT_ch")
                        nc.scalar.copy(out=hT_ch[:], in_=tr[:])
                        hT_chs[ch] = hT_ch
                    for (dst_sb, w_t, to_dram) in ((None, wl_sb[l], True), (xr_own, wr_sb[l], False)):
                        big = pmm.tile([P, 512], f32, tag="big")
                        for i, ch in enumerate(chunks):
                            nc.tensor.matmul(out=big[:, i * D:(i + 1) * D], lhsT=hT_chs[ch][:],
                                             rhs=w_t[:], start=True, stop=True)
                        ncols = len(chunks) * D
                        if to_dram:
                            stage = wp.tile([P, 512], f32, tag="xl_stage")
                            nc.scalar.copy(out=stage[:, :ncols], in_=big[:, :ncols])
                            # SBUF (p, i*D+d) -> DRAM row (16b+i)*128+p
                            dma = nc.sync.dma_start(
                                out=xl_own[:].rearrange("(a p) d -> p a d", p=P)[
                                    :, b * 16:b * 16 + len(chunks), :],
                                in_=stage[:, :ncols].rearrange("p (a d) -> p a d", d=D))
                            xl_dmas.append(dma)
                        else:
                            nc.scalar.copy(out=dst_sb[:, b * 16 * D: b * 16 * D + ncols],
                                           in_=big[:, :ncols])
                # (b) allgather xl -> xl_all; write pad-poison row
                if cfg.ncores > 1:
                    cc = nc.gpsimd.collective_compute(
                        "AllGather", mybir.AluOpType.bypass,
                        replica_groups=[list(range(cfg.ncores))],
                        ins=[xl_own[:]], outs=[xl_allg[:]])
                    for dma in xl_dmas:
                        add_dep_helper(cc.ins, dma.ins)
                    cpl = nc.sync.dma_start(out=xl_all[:], in_=xl_allg[:])
                    add_dep_helper(cpl.ins, cc.ins)
                else:
                    cpl = nc.sync.dma_start(out=xl_all[:], in_=xl_own[:])
                    for dma in xl_dmas:
                        add_dep_helper(cpl.ins, dma.ins)
                pv = nc.sync.dma_start(out=xl_all[cfg.padrow:cfg.padrow + 1, :], in_=padv_in[l][:])
                add_dep_helper(pv.ins, cpl.ins)

                # (c) edge phase per group
                for g in range(NGRP):
                    W = cfg.wg[g]
                    co = cfg.cumw[g]
                    idx_sb = ep.tile([P, W], i32, tag="idx")
                    nc.sync.dma_start(out=idx_sb[:], in_=idx_in[:, co:co + W])
                    # per-column [P,1] gathers: multi-offset indirect DMA is
                    # broken on HW (lane-sprayed offset fetch); one call per
                    # w-slot column is the verified-correct form.
                    xl_g = ep.tile([P, W * D], f32, tag="xl_g")
                    for w in range(W):
                        gth = nc.gpsimd.indirect_dma_start(
                            out=xl_g[:, w * D:(w + 1) * D], out_offset=None,
                            in_=xl_all[:],
                            in_offset=bass.IndirectOffsetOnAxis(ap=idx_sb[:, w:w + 1], axis=0))
                        add_dep_helper(gth.ins, pv.ins)
                    s_t = ep.tile([P, W * D], f32, tag="s_t")
                    nc.vector.tensor_tensor(
                        out=s_t[:].rearrange("p (w d) -> p w d", d=D),
                        in0=xl_g[:].rearrange("p (w d) -> p w d", d=D),
                        in1=xr_own[:, g * D:(g + 1) * D].unsqueeze(1).to_broadcast([P, W, D]),
                        op=mybir.AluOpType.add)
                    lr_t = ep.tile([P, W * D], f32, tag="lr_t")
                    nc.vector.scalar_tensor_tensor(
                        out=lr_t[:], in0=s_t[:], scalar=NEG_SLOPE, in1=s_t[:],
                        op0=mybir.AluOpType.mult, op1=mybir.AluOpType.max)
                    u_t = ep.tile([P, W * D], f32, tag="u_t")
                    nc.vector.tensor_tensor(out=u_t[:], in0=lr_t[:], in1=att_sb[:, :W * D],
                                            op=mybir.AluOpType.mult)
                    logit = ep.tile([P, W * H], f32, tag="logit")
                    nc.vector.tensor_reduce(
                        out=logit[:], in_=u_t[:].rearrange("p (wh c) -> p wh c", c=C),
                        axis=mybir.AxisListType.X, op=mybir.AluOpType.add)
                    ex_t = ep.tile([P, W * H], f32, tag="ex_t")
                    nc.scalar.activation(out=ex_t[:], in_=logit[:],
                                         func=mybir.ActivationFunctionType.Exp)
                    den = ep.tile([P, H], f32, tag="den")
                    nc.vector.tensor_reduce(
                        out=den[:], in_=ex_t[:].rearrange("p (w h) -> p h w", h=H),
                        axis=mybir.AxisListType.X, op=mybir.AluOpType.add)
                    dmx = ep.tile([P, H], f32, tag="dmx")
                    nc.vector.tensor_scalar_max(dmx[:], den[:], 1e-16)
                    rec = ep.tile([P, H], f32, tag="rec")
                    nc.vector.reciprocal(out=rec[:], in_=dmx[:])
                    tmp_t = ep.tile([P, W * D], f32, tag="tmp_t")
                    nc.vector.tensor_tensor(
                        out=tmp_t[:].rearrange("p (w h c) -> p w h c", h=H, c=C),
                        in0=xl_g[:].rearrange("p (w h c) -> p w h c", h=H, c=C),
                        in1=ex_t[:].rearrange("p (w h) -> p w h", h=H).unsqueeze(3).to_broadcast([P, W, H, C]),
                        op=mybir.AluOpType.mult)
                    numer = ep.tile([P, D], f32, tag="numer")
                    nc.vector.tensor_reduce(
                        out=numer[:], in_=tmp_t[:].rearrange("p (w d) -> p d w", d=D),
                        axis=mybir.AxisListType.X, op=mybir.AluOpType.add)
                    o1 = ep.tile([P, D], f32, tag="o1")
                    nc.vector.tensor_tensor(
                        out=o1[:].rearrange("p (h c) -> p h c", c=C),
                        in0=numer[:].rearrange("p (h c) -> p h c", c=C),
                        in1=rec[:].unsqueeze(2).to_broadcast([P, H, C]),
                        op=mybir.AluOpType.mult)
                    o3 = ep.tile([P, D], f32, tag="o3")
                    nc.vector.tensor_tensor(out=o3[:], in0=o1[:], in1=h_view(h_cur)[:, g, :],
                                            op=mybir.AluOpType.add)
                    nc.vector.tensor_tensor(out=h_view(h_nxt)[:, g, :], in0=o3[:], in1=cb_sb[l][:],
                                            op=mybir.AluOpType.add)
                nc.vector.tensor_copy(out=ones_col(h_nxt)[:, :, 0], in_=ones_sb[:].to_broadcast([P, NGRP]))
                h_cur, h_nxt = h_nxt, h_cur

            if dbg_h is not None:
                nc.sync.dma_start(out=dbg_h[:], in_=h_cur[:])

            # ---- P3: out = h2 @ w_pred + b_pred ----
            nbatch = math.ceil(NGRP / 16)
            for b in range(nbatch):
                chunks = range(b * 16, min((b + 1) * 16, NGRP))
                fin = pmm.tile([P, 16 * cfg.ncls], f32, tag="fin", bufs=1)
                for i, ch in enumerate(chunks):
                    tr = ptr.tile([HS, P], f32, tag="tr")
                    nc.tensor.transpose(
                        out=tr[:], in_=h_cur[:].rearrange("p (g s) -> p g s", s=HS)[:, ch, :],
                        identity=ident[:])
                    hT_ch = wp.tile([HS, P], f32, tag="hT_ch")
                    nc.scalar.copy(out=hT_ch[:], in_=tr[:])
                    nc.tensor.matmul(out=fin[:, i * cfg.ncls:(i + 1) * cfg.ncls], lhsT=hT_ch[:],
                                     rhs=wpred_sb[:], start=True, stop=True)
                ncols = len(chunks) * cfg.ncls
                ostage = wp.tile([P, 16 * cfg.ncls], f32, tag="ostage")
                nc.scalar.copy(out=ostage[:, :ncols], in_=fin[:, :ncols])
                nc.sync.dma_start(out=out_dram[:, b * 16 * cfg.ncls: b * 16 * cfg.ncls + ncols],
                                  in_=ostage[:, :ncols])
    nc.finalize()
    return nc


def assemble_output(cfg: Cfg, meta, core_outs):
    """core_outs: list of [128, NGRP*ncls] arrays -> full [n_real, ncls]."""
    full = np.zeros((cfg.npad, cfg.ncls), np.float32)
    for k in range(cfg.ncores):
        # out[p, g*ncls+j] holds node lid g*128+p
        o = core_outs[k].reshape(128, cfg.ngrp, cfg.ncls).transpose(1, 0, 2).reshape(cfg.nl, cfg.ncls)
        full[meta["nodes_by_lid"][k]] = o
    return full[:cfg.n_real]


_LAST = {}


def bench(inputs, iters=20) -> dict:
    """Correctness + repeat-execution timing via a hand-rolled PJRT runner
    (no NTFF hook in this environment). Returns output + per-iter seconds."""
    import time

    import jax
    from jax.sharding import Mesh, NamedSharding, PartitionSpec
    from jax.experimental.shard_map import shard_map
    import concourse.mybir as mybir
    from concourse import bass2jax
    from concourse.bass2jax import _bass_exec_p, install_neuronx_cc_hook, partition_id_tensor

    inputs = {k: np.asarray(v) for k, v in inputs.items()}
    cfg, per_core, shared, meta = host_prep(ncores=NCORES, **inputs)
    nc = build_program(cfg)
    in_maps = [{**shared, **pc} for pc in per_core]

    install_neuronx_cc_hook()
    partition_name = nc.partition_id_tensor.name if nc.partition_id_tensor else None
    in_names, out_names, out_avals, zero_outs = [], [], [], []
    for alloc in nc.m.functions[0].allocations:
        if not isinstance(alloc, mybir.MemoryLocationSet):
            continue
        name = alloc.memorylocations[0].name
        if alloc.kind == "ExternalInput":
            if name != partition_name:
                in_names.append(name)
        elif alloc.kind == "ExternalOutput":
            shape = tuple(alloc.tensor_shape)
            dtype = mybir.dt.np(alloc.dtype)
            out_names.append(name)
            out_avals.append(jax.core.ShapedArray(shape, dtype))
            zero_outs.append(np.zeros(shape, dtype))
    n_params = len(in_names)
    n_outs = len(out_avals)
    all_in_names = list(in_names) + list(out_names)
    if partition_name is not None:
        all_in_names.append(partition_name)
    donate = tuple(range(n_params, n_params + n_outs))

    def _body(*args):
        operands = list(args)
        if partition_name is not None:
            operands.append(partition_id_tensor())
        return tuple(_bass_exec_p.bind(
            *operands, out_avals=tuple(out_avals), in_names=tuple(all_in_names),
            out_names=tuple(out_names), lowering_input_output_aliases=(),
            sim_require_finite=True, sim_require_nnan=True, nc=nc))

    devices = jax.devices()[:NCORES]
    mesh = Mesh(np.asarray(devices), ("core",))
    in_specs = (PartitionSpec("core"),) * (n_params + n_outs)
    out_specs = (PartitionSpec("core"),) * n_outs
    sharded = jax.jit(shard_map(_body, mesh=mesh, in_specs=in_specs,
                                out_specs=out_specs, check_rep=False),
                      donate_argnums=donate, keep_unused=True)
    concat_in = [np.concatenate([np.asarray(in_maps[c][n]) for c in range(NCORES)], axis=0)
                 for n in in_names]
    t0 = time.time()
    sh = NamedSharding(mesh, PartitionSpec("core"))
    args_dev = [jax.device_put(a, sh) for a in concat_in]
    jax.block_until_ready(args_dev)
    t_put = time.time() - t0

    def zeros_dev():
        return [jax.device_put(np.zeros((NCORES * z.shape[0], *z.shape[1:]), z.dtype), sh)
                for z in zero_outs]

    t0 = time.time()
    outs = sharded(*args_dev, *zeros_dev())
    jax.block_until_ready(outs)
    t_first = time.time() - t0
    result = [
        {name: np.asarray(outs[i]).reshape(NCORES, *out_avals[i].shape)[c]
         for i, name in enumerate(out_names)} for c in range(NCORES)]

    # warm + timed loop (zeros pre-staged on device; donated per call)
    zsets = [zeros_dev() for _ in range(iters + 2)]
    jax.block_until_ready(zsets)
    r = sharded(*args_dev, *zsets[0])
    jax.block_until_ready(r)
    r = sharded(*args_dev, *zsets[1])
    jax.block_until_ready(r)
    t0 = time.time()
    rs = [sharded(*args_dev, *zsets[2 + i]) for i in range(iters)]
    jax.block_until_ready(rs)
    per_iter = (time.time() - t0) / iters

    out_full = assemble_output(cfg, meta, [result[k]["out"] for k in range(NCORES)])
    return {"out": out_full, "per_iter_s": per_iter, "first_s": t_first,
            "put_s": t_put, "cfg": cfg}


def kernel(**inputs) -> np.ndarray:
    import time

    from concourse.bass_utils import run_bass_kernel_spmd

    inputs = {k: np.asarray(v) for k, v in inputs.items()}
    cfg, per_core, shared, meta = host_prep(ncores=NCORES, **inputs)
    nc = build_program(cfg)
    in_maps = [{**shared, **pc} for pc in per_core]
    res = None
    for attempt in range(2):
        try:
            res = run_bass_kernel_spmd(nc, in_maps, core_ids=list(range(NCORES)))
            break
        except Exception:
            # transient device wedge (NRT_EXEC_UNIT_UNRECOVERABLE) recovers on
            # a fresh attempt once the runtime re-initializes
            if attempt == 1:
                raise
            time.sleep(20)
    assert res is not None
    _LAST["res"] = res
    _LAST["cfg"] = cfg
    outs = [res.results[k]["out"] for k in range(NCORES)]
    return assemble_output(cfg, meta, outs)

